# revision 9
# baseline (speedup 1.0000x reference)
"""Trainium2 Bass kernel for nn_LowBitMixIn.

Computes out[b,o,t] = sum_i mixer[o,i] * x[b, perm[i], t] for
x:[16,1024,4096] f32, mixer:[1024,1024] f32 (banded: 7 cyclic
sub-diagonals), perm:[1024] int32.

Strategy: data-parallel over batch (2 batches per core, 8 cores).
Host side builds, from the tiny mixer/permutation, per-output-tile
stationary matmul weights plus row-gather indices that fold the
permutation into an indirect DMA. The band (signed diagonal span S)
lets each output tile of M = 129-S rows be computed from a single
128-row gathered input window with ONE K=128 matmul per N-chunk.
If the mixer turns out not to be banded (verified by exact
reconstruction), a generic block-dense path is used instead.
"""

import os
import sys

import numpy as np

sys.path.insert(0, "/opt/trn_rl_repo")

from concourse import bacc, bass, mybir, tile  # noqa: E402
from concourse.bass_utils import run_bass_kernel_spmd  # noqa: E402

F = 1024
T = 4096
B = 16
N_CORES = 8
B_SHARD = B // N_CORES
NCHUNK = 512  # fp32 moving-operand max per matmul (one PSUM bank)

_PROGRAM_CACHE = {}
LAST_RESULTS = None  # test harness introspection (exec_time_ns etc.)
LAST_NC = None
LAST_IN_MAPS = None


def _build_banded_program(b_shard, f, t, tile_m, n_tiles, reps=1):
    nc = bacc.Bacc()
    x_in = nc.declare_dram_parameter("x", [b_shard * f, t], mybir.dt.float32, isOutput=False)
    wts_in = nc.declare_dram_parameter(
        "wts", [n_tiles, 128, tile_m], mybir.dt.float32, isOutput=False
    )
    gidx_in = nc.declare_dram_parameter(
        "gidx", [128, b_shard * n_tiles], mybir.dt.int32, isOutput=False
    )
    out_ext = nc.declare_dram_parameter(
        "out", [b_shard * f, t], mybir.dt.float32, isOutput=True
    )
    n_chunks = t // NCHUNK
    with tile.TileContext(nc) as tc:
        with (
            tc.tile_pool(name="const", bufs=1) as cpool,
            tc.tile_pool(name="xp", bufs=3) as xpool,
            tc.tile_pool(name="op", bufs=3) as opool,
            tc.tile_pool(name="ps", bufs=6, space="PSUM") as pspool,
        ):
            idx_tile = cpool.tile([128, b_shard * n_tiles], mybir.dt.int32)
            nc.sync.dma_start(out=idx_tile[:], in_=gidx_in[:])
            w_tiles = []
            for p in range(n_tiles):
                wt = cpool.tile([128, tile_m], mybir.dt.float32, tag=f"w{p}")
                nc.sync.dma_start(out=wt[:], in_=wts_in[p])
                w_tiles.append(wt)
            for _rep in range(reps):
                for bi in range(b_shard):
                    for p in range(n_tiles):
                        _emit_banded_tile(
                            nc, f, t, tile_m, bi, p, n_tiles,
                            idx_tile, w_tiles, xpool, opool, pspool,
                            x_in, out_ext,
                        )
    return nc


def _emit_banded_tile(
    nc, f, t, tile_m, bi, p, n_tiles,
    idx_tile, w_tiles, xpool, opool, pspool, x_in, out_ext,
):
    n_chunks = t // NCHUNK
    o0 = p * tile_m
    m_p = min(tile_m, f - o0)
    col = bi * n_tiles + p
    xt = xpool.tile([128, t], mybir.dt.float32, tag="xt")
    nc.gpsimd.indirect_dma_start(
        out=xt[:],
        out_offset=None,
        in_=x_in[:],
        in_offset=bass.IndirectOffsetOnAxis(
            ap=idx_tile[:, col : col + 1], axis=0
        ),
    )
    ot = opool.tile([128, t], mybir.dt.float32, tag="ot")
    for ni in range(n_chunks):
        ps = pspool.tile([128, NCHUNK], mybir.dt.float32)
        nc.tensor.matmul(
            out=ps[:m_p, :],
            lhsT=w_tiles[p][:, :m_p],
            rhs=xt[:, ni * NCHUNK : (ni + 1) * NCHUNK],
            start=True,
            stop=True,
        )
        nc.vector.tensor_copy(
            out=ot[:m_p, ni * NCHUNK : (ni + 1) * NCHUNK],
            in_=ps[:m_p, :],
        )
    nc.sync.dma_start(
        out=out_ext[bi * f + o0 : bi * f + o0 + m_p, :],
        in_=ot[:m_p, :],
    )


def _build_dense_program(b_shard, f, t):
    """Fallback: generic block matmul out_p = sum_q M[p,q] @ xp_q.

    Splits T in halves to fit 8 resident gathered input tiles in SBUF.
    """
    nc = bacc.Bacc()
    nq = f // 128
    x_in = nc.declare_dram_parameter("x", [b_shard * f, t], mybir.dt.float32, isOutput=False)
    wts_in = nc.declare_dram_parameter(
        "wts", [nq, nq, 128, 128], mybir.dt.float32, isOutput=False
    )
    gidx_in = nc.declare_dram_parameter(
        "gidx", [128, b_shard * nq], mybir.dt.int32, isOutput=False
    )
    out_ext = nc.declare_dram_parameter(
        "out", [b_shard * f, t], mybir.dt.float32, isOutput=True
    )
    t_half = t // 2
    n_chunks = t_half // NCHUNK
    with tile.TileContext(nc) as tc:
        with (
            tc.tile_pool(name="const", bufs=1) as cpool,
            tc.tile_pool(name="xp", bufs=10) as xpool,
            tc.tile_pool(name="op", bufs=2) as opool,
            tc.tile_pool(name="ps", bufs=6, space="PSUM") as pspool,
        ):
            idx_tile = cpool.tile([128, b_shard * nq], mybir.dt.int32)
            nc.sync.dma_start(out=idx_tile[:], in_=gidx_in[:])
            w_tiles = {}
            for p in range(nq):
                for q in range(nq):
                    wt = cpool.tile([128, 128], mybir.dt.float32, tag=f"w{p}_{q}")
                    nc.sync.dma_start(out=wt[:], in_=wts_in[p, q])
                    w_tiles[(p, q)] = wt
            for bi in range(b_shard):
                for th in range(2):
                    t0 = th * t_half
                    xts = []
                    for q in range(nq):
                        col = bi * nq + q
                        xt = xpool.tile([128, t_half], mybir.dt.float32, tag="xt")
                        nc.gpsimd.indirect_dma_start(
                            out=xt[:],
                            out_offset=None,
                            in_=x_in[:],
                            in_offset=bass.IndirectOffsetOnAxis(
                                ap=idx_tile[:, col : col + 1], axis=0
                            ),
                            element_offset=t0,
                        )
                        xts.append(xt)
                    for p in range(nq):
                        ot = opool.tile([128, t_half], mybir.dt.float32, tag="ot")
                        for ni in range(n_chunks):
                            ps = pspool.tile([128, NCHUNK], mybir.dt.float32)
                            for q in range(nq):
                                nc.tensor.matmul(
                                    out=ps[:, :],
                                    lhsT=w_tiles[(p, q)][:],
                                    rhs=xts[q][:, ni * NCHUNK : (ni + 1) * NCHUNK],
                                    start=(q == 0),
                                    stop=(q == nq - 1),
                                )
                            nc.vector.tensor_copy(
                                out=ot[:, ni * NCHUNK : (ni + 1) * NCHUNK],
                                in_=ps[:, :],
                            )
                        nc.sync.dma_start(
                            out=out_ext[
                                bi * f + p * 128 : bi * f + (p + 1) * 128,
                                t0 : t0 + t_half,
                            ],
                            in_=ot[:, :],
                        )
    return nc


def _analyze(mixer, permutation, b_shard, f):
    """Derive band structure + weights/indices. Returns (mode, tile_m,
    n_tiles, wts, gidx)."""
    perm = permutation.astype(np.int64)
    o_idx, c_idx = np.nonzero(mixer)
    if len(o_idx) == 0:
        d_lo = d_hi = 0
    else:
        d = (o_idx - c_idx) % f
        d_signed = np.where(d > f // 2, d - f, d)
        d_lo, d_hi = int(d_signed.min()), int(d_signed.max())
    span = d_hi - d_lo + 1
    if span <= 128:
        tile_m = 129 - span
        n_tiles = -(-f // tile_m)
        wts = np.zeros((n_tiles, 128, tile_m), np.float32)
        gidx = np.zeros((128, b_shard * n_tiles), np.int32)
        a_hat = np.zeros((f, f), np.float32)
        k_arange = np.arange(128)
        for p in range(n_tiles):
            o0 = p * tile_m
            m_p = min(tile_m, f - o0)
            rows = (o0 - d_hi + k_arange) % f  # feature index i per window row
            wts[p, :, :m_p] = mixer[np.ix_(range(o0, o0 + m_p), rows)].T
            a_hat[np.ix_(range(o0, o0 + m_p), rows)] = wts[p, :, :m_p].T
            for bi in range(b_shard):
                gidx[:, bi * n_tiles + p] = bi * f + perm[rows]
        if np.array_equal(a_hat, mixer):
            return ("banded", tile_m, n_tiles, wts, gidx)
    # dense fallback
    nq = f // 128
    wts = np.ascontiguousarray(
        mixer.reshape(nq, 128, nq, 128).transpose(0, 2, 3, 1), dtype=np.float32
    )
    gidx = np.zeros((128, b_shard * nq), np.int32)
    for bi in range(b_shard):
        for q in range(nq):
            gidx[:, bi * nq + q] = bi * f + perm[q * 128 : (q + 1) * 128]
    return ("dense", 128, nq, wts, gidx)


def kernel(x, mixer, permutation):
    global LAST_RESULTS
    x = np.ascontiguousarray(x, dtype=np.float32)
    mixer = np.asarray(mixer, dtype=np.float32)
    permutation = np.asarray(permutation)
    b, f, t = x.shape
    b_shard = b // N_CORES

    mode, tile_m, n_tiles, wts, gidx = _analyze(mixer, permutation, b_shard, f)

    key = (mode, b_shard, f, t, tile_m, n_tiles)
    if key not in _PROGRAM_CACHE:
        if mode == "banded":
            _PROGRAM_CACHE[key] = _build_banded_program(
                b_shard, f, t, tile_m, n_tiles
            )
        else:
            _PROGRAM_CACHE[key] = _build_dense_program(b_shard, f, t)
    nc = _PROGRAM_CACHE[key]
    if not getattr(nc, "_lowbit_compiled", False):
        nc.compile()
        nc._lowbit_compiled = True

    in_maps = []
    for i in range(N_CORES):
        in_maps.append(
            {
                "x": np.ascontiguousarray(
                    x[i * b_shard : (i + 1) * b_shard].reshape(b_shard * f, t)
                ),
                "wts": wts,
                "gidx": gidx,
            }
        )
    global LAST_NC, LAST_IN_MAPS
    LAST_NC = nc
    LAST_IN_MAPS = in_maps
    res = run_bass_kernel_spmd(nc, in_maps, list(range(N_CORES)))
    LAST_RESULTS = res
    out = np.concatenate(
        [r["out"].reshape(b_shard, f, t) for r in res.results], axis=0
    )
    return out


# revision 10
# speedup vs baseline: 32.1415x; 32.1415x over previous
"""Trainium2 Bass kernel for nn_LowBitMixIn.

Computes out[b,o,t] = sum_i mixer[o,i] * x[b, perm[i], t] for
x:[16,1024,4096] f32, mixer:[1024,1024] f32 (banded: 7 cyclic
sub-diagonals), perm:[1024] int32.

Strategy: data-parallel over batch (2 batches per core, 8 cores).
Host side builds, from the tiny mixer/permutation, per-output-tile
stationary matmul weights plus row-gather indices that fold the
permutation into an indirect DMA. The band (signed diagonal span S)
lets each output tile of M = 129-S rows be computed from a single
128-row gathered input window with ONE K=128 matmul per N-chunk.
If the mixer turns out not to be banded (verified by exact
reconstruction), a generic block-dense path is used instead.
"""

import os
import sys

import numpy as np

sys.path.insert(0, "/opt/trn_rl_repo")

from concourse import bacc, bass, mybir, tile  # noqa: E402
from concourse.bass_utils import run_bass_kernel_spmd  # noqa: E402

F = 1024
T = 4096
B = 16
N_CORES = 8
B_SHARD = B // N_CORES
NCHUNK = 512  # fp32 moving-operand max per matmul (one PSUM bank)

_PROGRAM_CACHE = {}
LAST_RESULTS = None  # test harness introspection (exec_time_ns etc.)
LAST_NC = None
LAST_IN_MAPS = None


def _build_banded_program(b_shard, f, t, tile_m, n_tiles, reps=1):
    nc = bacc.Bacc()
    x_in = nc.declare_dram_parameter("x", [b_shard * f, t], mybir.dt.float32, isOutput=False)
    wts_in = nc.declare_dram_parameter(
        "wts", [n_tiles, 128, tile_m], mybir.dt.float32, isOutput=False
    )
    gidx_in = nc.declare_dram_parameter(
        "gidx", [128, b_shard * n_tiles], mybir.dt.int32, isOutput=False
    )
    out_ext = nc.declare_dram_parameter(
        "out", [b_shard * f, t], mybir.dt.float32, isOutput=True
    )
    n_chunks = t // NCHUNK
    with tile.TileContext(nc) as tc:
        with (
            tc.tile_pool(name="const", bufs=1) as cpool,
            tc.tile_pool(name="xp", bufs=3) as xpool,
            tc.tile_pool(name="op", bufs=3) as opool,
            tc.tile_pool(name="ps", bufs=6, space="PSUM") as pspool,
        ):
            idx_tile = cpool.tile([128, b_shard * n_tiles], mybir.dt.int32)
            nc.sync.dma_start(out=idx_tile[:], in_=gidx_in[:])
            w_tiles = []
            for p in range(n_tiles):
                wt = cpool.tile([128, tile_m], mybir.dt.float32, tag=f"w{p}")
                nc.sync.dma_start(out=wt[:], in_=wts_in[p])
                w_tiles.append(wt)
            for _rep in range(reps):
                for bi in range(b_shard):
                    for p in range(n_tiles):
                        _emit_banded_tile(
                            nc, f, t, tile_m, bi, p, n_tiles,
                            idx_tile, w_tiles, xpool, opool, pspool,
                            x_in, out_ext,
                        )
    return nc


def _emit_banded_tile(
    nc, f, t, tile_m, bi, p, n_tiles,
    idx_tile, w_tiles, xpool, opool, pspool, x_in, out_ext,
):
    n_chunks = t // NCHUNK
    o0 = p * tile_m
    m_p = min(tile_m, f - o0)
    col = bi * n_tiles + p
    xt = xpool.tile([128, t], mybir.dt.float32, tag="xt")
    nc.gpsimd.indirect_dma_start(
        out=xt[:],
        out_offset=None,
        in_=x_in[:],
        in_offset=bass.IndirectOffsetOnAxis(
            ap=idx_tile[:, col : col + 1], axis=0
        ),
    )
    ot = opool.tile([128, t], mybir.dt.float32, tag="ot")
    for ni in range(n_chunks):
        ps = pspool.tile([128, NCHUNK], mybir.dt.float32)
        nc.tensor.matmul(
            out=ps[:m_p, :],
            lhsT=w_tiles[p][:, :m_p],
            rhs=xt[:, ni * NCHUNK : (ni + 1) * NCHUNK],
            start=True,
            stop=True,
        )
        nc.vector.tensor_copy(
            out=ot[:m_p, ni * NCHUNK : (ni + 1) * NCHUNK],
            in_=ps[:m_p, :],
        )
    # SWDGE (gpsimd) writes sustain ~2.7x the bandwidth of HWDGE writes on
    # this target; 4-way column split pipelines the write receipts.
    h = t // 4
    for ci in range(4):
        nc.gpsimd.dma_start(
            out=out_ext[
                bi * f + o0 : bi * f + o0 + m_p, ci * h : (ci + 1) * h
            ],
            in_=ot[:m_p, ci * h : (ci + 1) * h],
        )


def _build_dense_program(b_shard, f, t):
    """Fallback: generic block matmul out_p = sum_q M[p,q] @ xp_q.

    Splits T in halves to fit 8 resident gathered input tiles in SBUF.
    """
    nc = bacc.Bacc()
    nq = f // 128
    x_in = nc.declare_dram_parameter("x", [b_shard * f, t], mybir.dt.float32, isOutput=False)
    wts_in = nc.declare_dram_parameter(
        "wts", [nq, nq, 128, 128], mybir.dt.float32, isOutput=False
    )
    gidx_in = nc.declare_dram_parameter(
        "gidx", [128, b_shard * nq], mybir.dt.int32, isOutput=False
    )
    out_ext = nc.declare_dram_parameter(
        "out", [b_shard * f, t], mybir.dt.float32, isOutput=True
    )
    t_half = t // 2
    n_chunks = t_half // NCHUNK
    with tile.TileContext(nc) as tc:
        with (
            tc.tile_pool(name="const", bufs=1) as cpool,
            tc.tile_pool(name="xp", bufs=10) as xpool,
            tc.tile_pool(name="op", bufs=2) as opool,
            tc.tile_pool(name="ps", bufs=6, space="PSUM") as pspool,
        ):
            idx_tile = cpool.tile([128, b_shard * nq], mybir.dt.int32)
            nc.sync.dma_start(out=idx_tile[:], in_=gidx_in[:])
            w_tiles = {}
            for p in range(nq):
                for q in range(nq):
                    wt = cpool.tile([128, 128], mybir.dt.float32, tag=f"w{p}_{q}")
                    nc.sync.dma_start(out=wt[:], in_=wts_in[p, q])
                    w_tiles[(p, q)] = wt
            for bi in range(b_shard):
                for th in range(2):
                    t0 = th * t_half
                    xts = []
                    for q in range(nq):
                        col = bi * nq + q
                        xt = xpool.tile([128, t_half], mybir.dt.float32, tag="xt")
                        nc.gpsimd.indirect_dma_start(
                            out=xt[:],
                            out_offset=None,
                            in_=x_in[:],
                            in_offset=bass.IndirectOffsetOnAxis(
                                ap=idx_tile[:, col : col + 1], axis=0
                            ),
                            element_offset=t0,
                        )
                        xts.append(xt)
                    for p in range(nq):
                        ot = opool.tile([128, t_half], mybir.dt.float32, tag="ot")
                        for ni in range(n_chunks):
                            ps = pspool.tile([128, NCHUNK], mybir.dt.float32)
                            for q in range(nq):
                                nc.tensor.matmul(
                                    out=ps[:, :],
                                    lhsT=w_tiles[(p, q)][:],
                                    rhs=xts[q][:, ni * NCHUNK : (ni + 1) * NCHUNK],
                                    start=(q == 0),
                                    stop=(q == nq - 1),
                                )
                            nc.vector.tensor_copy(
                                out=ot[:, ni * NCHUNK : (ni + 1) * NCHUNK],
                                in_=ps[:, :],
                            )
                        nc.sync.dma_start(
                            out=out_ext[
                                bi * f + p * 128 : bi * f + (p + 1) * 128,
                                t0 : t0 + t_half,
                            ],
                            in_=ot[:, :],
                        )
    return nc


def _analyze(mixer, permutation, b_shard, f):
    """Derive band structure + weights/indices. Returns (mode, tile_m,
    n_tiles, wts, gidx)."""
    perm = permutation.astype(np.int64)
    o_idx, c_idx = np.nonzero(mixer)
    if len(o_idx) == 0:
        d_lo = d_hi = 0
    else:
        d = (o_idx - c_idx) % f
        d_signed = np.where(d > f // 2, d - f, d)
        d_lo, d_hi = int(d_signed.min()), int(d_signed.max())
    span = d_hi - d_lo + 1
    if span <= 128:
        tile_m = 129 - span
        n_tiles = -(-f // tile_m)
        wts = np.zeros((n_tiles, 128, tile_m), np.float32)
        gidx = np.zeros((128, b_shard * n_tiles), np.int32)
        a_hat = np.zeros((f, f), np.float32)
        k_arange = np.arange(128)
        for p in range(n_tiles):
            o0 = p * tile_m
            m_p = min(tile_m, f - o0)
            rows = (o0 - d_hi + k_arange) % f  # feature index i per window row
            wts[p, :, :m_p] = mixer[np.ix_(range(o0, o0 + m_p), rows)].T
            a_hat[np.ix_(range(o0, o0 + m_p), rows)] = wts[p, :, :m_p].T
            for bi in range(b_shard):
                gidx[:, bi * n_tiles + p] = bi * f + perm[rows]
        if np.array_equal(a_hat, mixer):
            return ("banded", tile_m, n_tiles, wts, gidx)
    # dense fallback
    nq = f // 128
    wts = np.ascontiguousarray(
        mixer.reshape(nq, 128, nq, 128).transpose(0, 2, 3, 1), dtype=np.float32
    )
    gidx = np.zeros((128, b_shard * nq), np.int32)
    for bi in range(b_shard):
        for q in range(nq):
            gidx[:, bi * nq + q] = bi * f + perm[q * 128 : (q + 1) * 128]
    return ("dense", 128, nq, wts, gidx)


def kernel(x, mixer, permutation):
    global LAST_RESULTS
    x = np.ascontiguousarray(x, dtype=np.float32)
    mixer = np.asarray(mixer, dtype=np.float32)
    permutation = np.asarray(permutation)
    b, f, t = x.shape
    b_shard = b // N_CORES

    mode, tile_m, n_tiles, wts, gidx = _analyze(mixer, permutation, b_shard, f)

    key = (mode, b_shard, f, t, tile_m, n_tiles)
    if key not in _PROGRAM_CACHE:
        if mode == "banded":
            _PROGRAM_CACHE[key] = _build_banded_program(
                b_shard, f, t, tile_m, n_tiles
            )
        else:
            _PROGRAM_CACHE[key] = _build_dense_program(b_shard, f, t)
    nc = _PROGRAM_CACHE[key]
    if not getattr(nc, "_lowbit_compiled", False):
        nc.compile()
        nc._lowbit_compiled = True

    in_maps = []
    for i in range(N_CORES):
        in_maps.append(
            {
                "x": np.ascontiguousarray(
                    x[i * b_shard : (i + 1) * b_shard].reshape(b_shard * f, t)
                ),
                "wts": wts,
                "gidx": gidx,
            }
        )
    global LAST_NC, LAST_IN_MAPS
    LAST_NC = nc
    LAST_IN_MAPS = in_maps
    res = run_bass_kernel_spmd(nc, in_maps, list(range(N_CORES)))
    LAST_RESULTS = res
    out = np.concatenate(
        [r["out"].reshape(b_shard, f, t) for r in res.results], axis=0
    )
    return out
